# revision 41
# baseline (speedup 1.0000x reference)
"""Multi-head attention (B=2, S=2048, D=1024, H=16, Dk=64) on 8 TRN2 cores.

Sharding: tensor-parallel on heads - 2 heads (dh=128 columns of the QKV
projections) per core.  Each core:
  1. projects qT/kT/vT = (W_slice.T @ x.T) for its 2 heads    [128, 4096]
     (bias folded into the PSUM->SBUF evacuation via tensor_scalar_add)
  2. transposes vT into per-(b,h) [j, d] blocks with an appended
     ones-column (so P@V_aug also yields the softmax row-sums)
  3. attention in mini-passes of (h, 512-query block): score tiles are
     [128 keys, 1024] spanning two key-blocks so each Exp activation
     covers 1024 columns; S->exp->PV is software-pipelined one jt-pair
     ahead so the PE never waits on the scalar engine
  4. normalizes with reciprocal_approx_fast + gpsimd partition_broadcast
  5. out-projection per 512-column block interleaved into the attention
     stream; partials DMA out as fp16
Host sums the 8 partialT outputs, adds bo, and transposes back.

All matmuls are fp16 with fp32 PSUM accumulation.
"""

import numpy as np

D = 1024
NTOK = 4096  # B * S
B = 2
S = 2048
DH = 128  # head-dim block per core (2 heads x 64)
N_CORES = 8

_CACHE = {}


def _build_nc(mm_dtype="float16"):
    import concourse.bacc as bacc
    import concourse.mybir as mybir
    import concourse.tile as tile

    dt = mybir.dt
    f32 = dt.float32
    mmdt = getattr(dt, mm_dtype)
    AF = mybir.ActivationFunctionType

    nc = bacc.Bacc("TRN2", target_bir_lowering=False, debug=False)

    xq = nc.dram_tensor("xq", [D, NTOK], mmdt, kind="ExternalInput").ap()
    xk = nc.dram_tensor("xk", [D, NTOK], mmdt, kind="ExternalInput").ap()
    xv = nc.dram_tensor("xv", [D, NTOK], mmdt, kind="ExternalInput").ap()
    wq = nc.dram_tensor("wq", [128, D], mmdt, kind="ExternalInput").ap()
    wk = nc.dram_tensor("wk", [128, D], mmdt, kind="ExternalInput").ap()
    wv = nc.dram_tensor("wv", [128, D], mmdt, kind="ExternalInput").ap()
    wo = nc.dram_tensor("wo", [128, D], mmdt, kind="ExternalInput").ap()
    bq = nc.dram_tensor("bq", [128, 1], f32, kind="ExternalInput").ap()
    bk = nc.dram_tensor("bk", [128, 1], f32, kind="ExternalInput").ap()
    bv = nc.dram_tensor("bv", [128, 1], f32, kind="ExternalInput").ap()
    c_ident = nc.dram_tensor("c_ident", [128, 64], mmdt, kind="ExternalInput").ap()
    pout = nc.dram_tensor("pout", [D, NTOK], mmdt, kind="ExternalOutput").ap()

    with tile.TileContext(nc) as tc:
        from contextlib import ExitStack

        with ExitStack() as stk:
            const = stk.enter_context(tc.tile_pool(name="const", bufs=1))
            wpool = stk.enter_context(tc.tile_pool(name="w", bufs=1))
            big = stk.enter_context(tc.tile_pool(name="big", bufs=1))
            xpool = stk.enter_context(tc.tile_pool(name="xt", bufs=16))
            xvh = stk.enter_context(tc.tile_pool(name="xvh", bufs=16))
            ptp = stk.enter_context(tc.tile_pool(name="pt", bufs=3))
            stp = stk.enter_context(tc.tile_pool(name="st", bufs=4))
            rsp = stk.enter_context(tc.tile_pool(name="rs", bufs=4))

            # ---- weights / biases / constants (k first: attention waits on kT+qT) ----
            wk_sb = wpool.tile([128, D], mmdt)
            wq_sb = wpool.tile([128, D], mmdt)
            wv_sb = wpool.tile([128, D], mmdt)
            wo_sb = wpool.tile([128, D], mmdt)
            bk_sb = const.tile([128, 1], f32)
            bq_sb = const.tile([128, 1], f32)
            bv_sb = const.tile([128, 1], f32)
            ident = const.tile([128, 64], mmdt)
            # weights/biases on the (idle) ACT hwdge queue so the sync queue
            # starts streaming x tiles immediately
            nc.scalar.dma_start(out=wk_sb, in_=wk)
            nc.scalar.dma_start(out=bk_sb, in_=bk)
            nc.scalar.dma_start(out=wq_sb, in_=wq)
            nc.scalar.dma_start(out=bq_sb, in_=bq)
            nc.scalar.dma_start(out=wv_sb, in_=wv)
            nc.scalar.dma_start(out=bv_sb, in_=bv)
            nc.scalar.dma_start(out=wo_sb, in_=wo)
            nc.scalar.dma_start(out=ident, in_=c_ident)
            # dummy exp: pulls the ACT_TABLE_LOAD into the (idle) proj phase
            warm = const.tile([128, 1], f32)
            nc.scalar.activation(warm, bk_sb, AF.Exp)

            # ---- persistent activations ----
            qT = big.tile([128, NTOK], mmdt)  # [dh, tok]
            kT = big.tile([128, NTOK], mmdt)
            vT = big.tile([128, NTOK], mmdt)
            v_sb = big.tile([128, 4 * 16 * 65], mmdt)  # [j, (b,h)*jt*(64+1)]
            oT = big.tile([128, NTOK], mmdt)  # [dh, tok] normalized context

            # ones columns: memset whole tile; v blocks overwritten by transposes
            nc.vector.memset(v_sb, 1.0)
            v_r = v_sb.rearrange("p (t c) -> p t c", c=65)

            PROJS = {
                "k": (xk, wk_sb, bk_sb, kT),
                "q": (xq, wq_sb, bq_sb, qT),
                "v": (xv, wv_sb, bv_sb, vT),
            }
            x_tiles = {}

            def emit_x_dma(name, b, kk):
                x_dram = PROJS[name][0]
                x_t = xpool.tile([128, 2048], mmdt, tag="xt", name=f"x{name}{b}_{kk}")
                nc.sync.dma_start(
                    out=x_t,
                    in_=x_dram[kk * 128 : (kk + 1) * 128, b * 2048 : (b + 1) * 2048],
                )
                x_tiles[(name, b, kk)] = x_t

            def emit_xv_half_dma(b, kk, half):
                x_t = xvh.tile([128, 1024], mmdt, tag="xvh", name=f"xv{b}_{kk}_{half}")
                col = b * 2048 + half * 1024
                nc.sync.dma_start(
                    out=x_t, in_=xv[kk * 128 : (kk + 1) * 128, col : col + 1024]
                )
                x_tiles[("v", b, kk, half)] = x_t

            def emit_proj_one(b, name):
                _, w_sb, b_sb, dst = PROJS[name]
                with tc.tile_pool(name=f"pp{name}{b}", bufs=4, space="PSUM") as pp:
                    acc = [
                        pp.tile([128, 512], f32, tag="pp", name=f"acc{name}{b}_{nn}")
                        for nn in range(4)
                    ]
                    for kk in range(8):
                        x_t = x_tiles[(name, b, kk)]
                        for nn in range(4):
                            nc.tensor.matmul(
                                acc[nn],
                                lhsT=w_sb[:, kk * 128 : (kk + 1) * 128],
                                rhs=x_t[:, nn * 512 : (nn + 1) * 512],
                                start=(kk == 0),
                                stop=(kk == 7),
                            )
                    for nn in range(4):
                        col = b * 2048 + nn * 512
                        # evac + bias add (per-partition scalar) + fp16 cast
                        nc.vector.tensor_scalar_add(
                            dst[:, col : col + 512], acc[nn], b_sb
                        )

            def proj_thunks(b, opj):
                """proj matmuls as paced thunks (nn-block at a time) sharing the
                outproj PSUM slot; interleaved into the other batch's attention
                to keep the PE continuously fed"""
                thunks = []
                for name in ("k", "q", "v"):
                    _, w_sb, b_sb, dst = PROJS[name]
                    is_v = name == "v"
                    for nn in range(4):
                        cell = {}
                        half, sub = nn // 2, nn % 2

                        def rhs_for(kk, name=name, nn=nn, half=half, sub=sub):
                            if name == "v":
                                return x_tiles[("v", b, kk, half)][
                                    :, sub * 512 : (sub + 1) * 512
                                ]
                            return x_tiles[(name, b, kk)][:, nn * 512 : (nn + 1) * 512]

                        def t_open(name=name, nn=nn, cell=cell, w_sb=w_sb, rhs_for=rhs_for):
                            cell["acc"] = opj.tile(
                                [128, 512], f32, tag="opj", name=f"pa{name}{b}_{nn}"
                            )
                            nc.tensor.matmul(
                                cell["acc"],
                                lhsT=w_sb[:, 0:128],
                                rhs=rhs_for(0),
                                start=True,
                                stop=False,
                            )

                        need0 = ("v", b, 0, half) if is_v else (name, b, 0)
                        thunks.append((need0, None, t_open))
                        for kk in range(1, 8):

                            def t_mm(kk=kk, cell=cell, w_sb=w_sb, rhs_for=rhs_for):
                                nc.tensor.matmul(
                                    cell["acc"],
                                    lhsT=w_sb[:, kk * 128 : (kk + 1) * 128],
                                    rhs=rhs_for(kk),
                                    start=False,
                                    stop=(kk == 7),
                                )

                            needk = ("v", b, kk, half) if is_v else (name, b, kk)
                            thunks.append((needk, None, t_mm))

                        def t_evac(name=name, nn=nn, cell=cell, b_sb=b_sb, dst=dst):
                            col = b * 2048 + nn * 512
                            nc.vector.tensor_scalar_add(
                                dst[:, col : col + 512], cell["acc"], b_sb
                            )

                        thunks.append((None, None, t_evac))
                return thunks

            def v0_thunks(opj):
                """batch-0 v-projection + transposes as thunks so attention
                starts before xv lands; group g covers token cols g*512+"""
                _, wv_sb_, bv_sb_, _ = PROJS["v"]
                thunks = []
                for g in range(4):
                    cell = {}

                    half, sub = g // 2, g % 2

                    def t_open(g=g, cell=cell, half=half, sub=sub):
                        cell["acc"] = opj.tile(
                            [128, 512], f32, tag="opj", name=f"pav0_{g}"
                        )
                        nc.tensor.matmul(
                            cell["acc"],
                            lhsT=wv_sb_[:, 0:128],
                            rhs=x_tiles[("v", 0, 0, half)][:, sub * 512 : (sub + 1) * 512],
                            start=True,
                            stop=False,
                        )

                    thunks.append((None, None, t_open))
                    for kk in range(1, 8):

                        def t_mm(g=g, kk=kk, cell=cell, half=half, sub=sub):
                            nc.tensor.matmul(
                                cell["acc"],
                                lhsT=wv_sb_[:, kk * 128 : (kk + 1) * 128],
                                rhs=x_tiles[("v", 0, kk, half)][
                                    :, sub * 512 : (sub + 1) * 512
                                ],
                                start=False,
                                stop=(kk == 7),
                            )

                        thunks.append((None, None, t_mm))

                    def t_evac(g=g, cell=cell):
                        nc.vector.tensor_scalar_add(
                            vT[:, g * 512 : (g + 1) * 512], cell["acc"], bv_sb_
                        )

                    thunks.append((None, None, t_evac))
                    for h in range(2):

                        def t_tr(g=g, h=h):
                            emit_transp_group(opj, 0, h, g)

                        thunks.append((None, ("tr", h, g), t_tr))
                return thunks

            def ensure_transp(state, h, g):
                """pop thunks until transpose (h, g) has been emitted"""
                if ("tr", h, g) in state["done"]:
                    return
                while state["proj"]:
                    need, label, fn = state["proj"].pop(0)
                    fn()
                    if label is not None:
                        state["done"].add(label)
                    if label == ("tr", h, g):
                        return

            def pop_proj(state, budget=2):
                for _ in range(budget):
                    if not state["proj"]:
                        return
                    need, label, fn = state["proj"][0]
                    if need is not None and need not in x_tiles:
                        return  # x tile not prefetched yet
                    state["proj"].pop(0)
                    fn()
                    if label is not None:
                        state["done"].add(label)

            def emit_transp_group(opj, b, h, g):
                """one group of 4 v-transpose j-tiles for (b, h).  The tile is
                [128,1024] fp16 (2KB) so it shares the outproj pool's tag slot
                ([128,512] f32 is the same byte size); only cols 0:256 used."""
                bh = b * 2 + h
                tp = opj.tile([128, 1024], mmdt, tag="opj", name=f"tp{bh}_{g}")
                for u in range(4):
                    jb = g * 4 + u
                    nc.tensor.transpose(
                        tp[:, u * 64 : (u + 1) * 64],
                        vT[
                            h * 64 : (h + 1) * 64,
                            b * 2048 + jb * 128 : b * 2048 + (jb + 1) * 128,
                        ],
                        ident[h * 64 : (h + 1) * 64, :],
                    )
                tp_r = tp[:, 0:256].rearrange("p (t c) -> p t c", c=64)
                nc.vector.tensor_copy(
                    v_r[:, bh * 16 + g * 4 : bh * 16 + g * 4 + 4, 0:64], tp_r
                )

            def emit_finalize(o_ps, b, h, q):
                """oT[h rows, q cols] = o_unnorm * broadcast(1/rowsum)"""
                i0 = b * 2048 + q * 512
                # custom-DVE recip mis-reads partition offsets: stage the rowsum
                # row into an SBUF tile at partition 0 first
                rsum = rsp.tile([1, 512], f32, tag="rsum", name=f"rsum{b}{h}{q}")
                nc.vector.tensor_copy(rsum, o_ps[64:65, :])
                rinv = rsp.tile([1, 512], f32, tag="ri", name=f"ri{b}{h}{q}")
                nc.vector.reciprocal_approx_fast(rinv, rsum)
                Rs = rsp.tile([64, 512], f32, tag="rs", name=f"Rs{b}{h}{q}")
                nc.gpsimd.partition_broadcast(Rs, rinv)
                nc.vector.tensor_mul(
                    oT[h * 64 : (h + 1) * 64, i0 : i0 + 512], o_ps[0:64, :], Rs
                )

            def emit_outproj_one(opj, b, q, dt_, evac_act=False):
                """one 128-row slice of partialT[:, q cols] -> fp16 DMA out"""
                col = b * 2048 + q * 512
                op = opj.tile([128, 512], f32, tag="opj", name=f"op{b}{q}_{dt_}")
                nc.tensor.matmul(
                    op,
                    lhsT=wo_sb[:, dt_ * 128 : (dt_ + 1) * 128],
                    rhs=oT[:, col : col + 512],
                    start=True,
                    stop=True,
                )
                st = stp.tile([128, 512], mmdt, tag="st", name=f"st{b}{q}_{dt_}")
                if evac_act:
                    nc.scalar.copy(st, op)
                else:
                    nc.vector.tensor_copy(st, op)
                nc.sync.dma_start(
                    out=pout[dt_ * 128 : (dt_ + 1) * 128, col : col + 512],
                    in_=st,
                )

            def emit_pv(o_ps, b, h, t, pt):
                bh = b * 2 + h
                for c in range(2):
                    jt = 2 * t + c
                    nc.tensor.matmul(
                        o_ps,
                        lhsT=v_sb[:, (bh * 16 + jt) * 65 : (bh * 16 + jt + 1) * 65],
                        rhs=pt[:, c * 512 : (c + 1) * 512],
                        start=(jt == 0),
                        stop=(jt == 15),
                    )

            def emit_pass(scp, opp, opj, b, h, q, state, first=False):
                """One (h, 512-query-block) mini-pass, software-pipelined."""
                i0 = b * 2048 + q * 512
                o_ps = opp.tile([65, 512], f32, tag="ops", name=f"o{b}{h}{q}")
                pts = []
                for t in range(8):
                    if first and b == 1:
                        # fold b1's v-transposes into its pass 0
                        emit_transp_group(opj, b, t // 4, t % 4)
                    sc = scp.tile([128, 1024], f32, tag="sc", name=f"s{b}{h}{q}_{t}")
                    for c in range(2):
                        jt = 2 * t + c
                        nc.tensor.matmul(
                            sc[:, c * 512 : (c + 1) * 512],
                            lhsT=kT[
                                h * 64 : (h + 1) * 64,
                                b * 2048 + jt * 128 : b * 2048 + (jt + 1) * 128,
                            ],
                            rhs=qT[h * 64 : (h + 1) * 64, i0 : i0 + 512],
                            start=True,
                            stop=True,
                        )
                    pt = ptp.tile([128, 1024], mmdt, tag="pt", name=f"p{b}{h}{q}_{t}")
                    nc.scalar.activation(pt, sc, AF.Exp, scale=0.125)
                    pts.append(pt)
                    # filler is split around the PV pair: outproj between
                    # exp and PV covers the exp latency; proj thunks after PV
                    # cover the next S-pair's WAR wait on exp(t-1)
                    for _ in range(2):
                        if state["out"]:
                            emit_outproj_one(opj, *state["out"].pop(0))
                    if t >= 1:
                        if b == 0 and q == 0:
                            ensure_transp(state, h, (t - 1) // 2)
                        emit_pv(o_ps, b, h, t - 1, pts[t - 1])
                    pop_proj(state)
                    if t == 1 and state["fin"] is not None:
                        fo, fb, fh, fq = state["fin"]
                        emit_finalize(fo, fb, fh, fq)
                        state["fin"] = None
                        if fh == 1:
                            state["out"].extend((fb, fq, dt_) for dt_ in range(8))
                    for _ in range(2):
                        if state["prefetch"]:
                            state["prefetch"].pop(0)()
                if b == 0 and q == 0:
                    ensure_transp(state, h, 3)
                emit_pv(o_ps, b, h, 7, pts[7])
                state["fin"] = (o_ps, b, h, q)

            def emit_attention(b, prefetch, carry_out, proj_b=None, with_v0=False):
                """carry_out: outproj work deferred from the previous batch,
                emitted during this batch's warm-up pass.  proj_b: batch whose
                projections are interleaved into this attention as PE filler.
                Returns this batch's final-q outproj specs for the next batch."""
                state = {
                    "fin": None,
                    "prefetch": prefetch,
                    "out": list(carry_out),
                    "done": set(),
                }
                with (
                    tc.tile_pool(name=f"sc{b}", bufs=2, space="PSUM") as scp,
                    tc.tile_pool(name=f"ops{b}", bufs=2, space="PSUM") as opp,
                    tc.tile_pool(name=f"opj{b}", bufs=2, space="PSUM") as opj,
                ):
                    state["proj"] = (v0_thunks(opj) if with_v0 else []) + (
                        proj_thunks(proj_b, opj) if proj_b is not None else []
                    )
                    for q in range(4):
                        for h in range(2):
                            emit_pass(
                                scp, opp, opj, b, h, q, state,
                                first=(q == 0 and h == 0),
                            )
                    # drain: finalize for (h1, q3); outproj deferred to next
                    # batch's warm-up pass when there is one
                    fo, fb, fh, fq = state["fin"]
                    emit_finalize(fo, fb, fh, fq)
                    while state["prefetch"]:
                        state["prefetch"].pop(0)()
                    while state["proj"]:
                        state["proj"].pop(0)[2]()
                    for i, spec in enumerate(state["out"]):
                        emit_outproj_one(opj, *spec, evac_act=(i % 2 == 1))
                    tail = [(fb, fq, dt_) for dt_ in range(8)]
                    if b == 1:
                        for i, spec in enumerate(tail):
                            emit_outproj_one(opj, *spec, evac_act=(i % 2 == 1))
                        tail = []
                    return tail

            # =========== emission schedule ===========
            for name in ("k", "q"):
                for kk in range(8):
                    emit_x_dma(name, 0, kk)
            for half in range(2):
                for kk in range(8):
                    emit_xv_half_dma(0, kk, half)
            emit_proj_one(0, "k")
            emit_proj_one(0, "q")

            prefetch = [
                (lambda n=n, kk=kk: emit_x_dma(n, 1, kk))
                for n in ("k", "q")
                for kk in range(8)
            ] + [
                (lambda kk=kk, half=half: emit_xv_half_dma(1, kk, half))
                for half in range(2)
                for kk in range(8)
            ]
            carry = emit_attention(0, prefetch, [], proj_b=1, with_v0=True)
            emit_attention(1, [], carry)

    nc.compile()
    return nc


MM_DTYPE = "float16"


def _get_nc():
    key = ("nc", MM_DTYPE)
    if key not in _CACHE:
        _CACHE[key] = _build_nc(MM_DTYPE)
    return _CACHE[key]


def _ensure_ntff_hook():
    """Register the NTFF profile hook module if the image lacks it."""
    import sys
    import types

    if "antenv.axon_hooks" in sys.modules:
        return
    try:
        from trn_agent_boot.trn_boot import _ntff_profile_via_ctypes
    except Exception:
        return
    hook = None
    try:
        hook = _ntff_profile_via_ctypes("/opt/axon/libaxon_pjrt.so")
    except Exception:
        hook = None
    mod = types.ModuleType("antenv.axon_hooks")
    mod._hook = hook
    mod.get_axon_ntff_profile_hook = lambda: mod._hook
    mod.set_axon_ntff_profile_hook = lambda h: setattr(mod, "_hook", h)
    sys.modules["antenv.axon_hooks"] = mod


def _run(inputs, trace=False):
    from concourse import bass_utils

    if trace:
        _ensure_ntff_hook()

    nc = _get_nc()
    query = np.asarray(inputs["query"], np.float32)
    key = np.asarray(inputs["key"], np.float32)
    value = np.asarray(inputs["value"], np.float32)
    Wq = np.asarray(inputs["Wq"], np.float32)
    Wk = np.asarray(inputs["Wk"], np.float32)
    Wv = np.asarray(inputs["Wv"], np.float32)
    Wo = np.asarray(inputs["Wo"], np.float32)
    bq = np.asarray(inputs["bq"], np.float32)
    bk = np.asarray(inputs["bk"], np.float32)
    bv = np.asarray(inputs["bv"], np.float32)
    bo = np.asarray(inputs["bo"], np.float32)

    if MM_DTYPE == "bfloat16":
        import ml_dtypes

        ext_dt = ml_dtypes.bfloat16
    elif MM_DTYPE == "float16":
        ext_dt = np.float16
    else:
        ext_dt = np.float32

    xqT = np.ascontiguousarray(query.reshape(NTOK, D).T.astype(ext_dt))
    xkT = np.ascontiguousarray(key.reshape(NTOK, D).T.astype(ext_dt))
    xvT = np.ascontiguousarray(value.reshape(NTOK, D).T.astype(ext_dt))

    def pack_w(Wc):
        return np.ascontiguousarray(
            Wc.reshape(8, 128, 128).transpose(1, 0, 2).reshape(128, D).astype(ext_dt)
        )

    ident_np = np.zeros((128, 64), np.float32)
    ident_np[np.arange(64), np.arange(64)] = 1.0
    ident_np[64 + np.arange(64), np.arange(64)] = 1.0
    consts = {
        "c_ident": np.ascontiguousarray(ident_np.astype(ext_dt)),
    }
    in_maps = []
    for c in range(N_CORES):
        sl = slice(c * 128, (c + 1) * 128)
        in_maps.append(
            {
                **consts,
                "xq": xqT,
                "xk": xkT,
                "xv": xvT,
                "wq": pack_w(Wq[:, sl]),
                "wk": pack_w(Wk[:, sl]),
                "wv": pack_w(Wv[:, sl]),
                "wo": np.ascontiguousarray(Wo[sl, :].astype(ext_dt)),
                "bq": np.ascontiguousarray(bq[sl].reshape(128, 1).astype(np.float32)),
                "bk": np.ascontiguousarray(bk[sl].reshape(128, 1).astype(np.float32)),
                "bv": np.ascontiguousarray(bv[sl].reshape(128, 1).astype(np.float32)),
            }
        )

    res = bass_utils.run_bass_kernel_spmd(
        nc, in_maps, core_ids=list(range(N_CORES)), trace=trace
    )
    outT = np.zeros((D, NTOK), np.float64)
    for c in range(N_CORES):
        outT += np.asarray(res.results[c]["pout"], np.float64)
    out = (outT.T + bo.astype(np.float64)).astype(np.float32)
    return out.reshape(B, S, D), res


def kernel(**inputs):
    out, _ = _run(inputs, trace=False)
    return out
